# revision 3
# baseline (speedup 1.0000x reference)
"""Binary-weight dense layer on 8 TRN2 NeuronCores — single-pass bf16, v3.

Computes out = x @ sign(W) + b for x:[8192,4096] f32, W:[4096,4096] f32,
b:[4096] f32, sharded row-wise over x (each core computes a [1024, 4096]
slice of the output; no collectives).

Per-core kernel strategy:
  - single bf16 matmul pass (sign(W) exact in bf16; x rounding ~2^-9 →
    ~1.7e-3 output rel err vs the 2e-2 gate).
  - prep: x tiles are cast f32->bf16 *during* the SWDGE load DMA, then one
    3D-out xbar transpose per [128 x 1024] tile scatters its eight k-tile
    stripes straight into the SBUF-resident lhsT layout (SBUF->SBUF, no
    DRAM staging roundtrip - prep HBM traffic is just the 16 MB x read).
  - engine split so nothing queues behind paced prep triggers: SP ring
    carries b + transposes, ACT ring carries W loads + sign quantization,
    SWDGE carries cast-loads + out writes, DVE does bias-add evictions.
  - main loop: for each n-slice (512 cols) keep 8 PSUM banks (one per
    m-tile) accumulating over all 32 k-tiles.
"""

import sys

if "/opt/trn_rl_repo" not in sys.path:
    sys.path.insert(0, "/opt/trn_rl_repo")

import numpy as np

import concourse.bass as bass
import concourse.mybir as mybir
import concourse.tile as tile
from concourse import bacc
from concourse.bass_utils import run_bass_kernel_spmd

N_CORES = 8
P = 128

B, N_IN, N_UNITS = 8192, 4096, 4096
M_SH = B // N_CORES  # 1024 rows of x per core

F32 = mybir.dt.float32
BF16 = mybir.dt.bfloat16


def build_module(m_sh=M_SH, k_dim=N_IN, n_dim=N_UNITS, reps=1, timing=False, do_prep=1):
    nc = bacc.Bacc("TRN2", target_bir_lowering=False, debug=False)

    x_in = nc.dram_tensor("x", [m_sh, k_dim], F32, kind="ExternalInput")
    w_in = nc.dram_tensor("W", [k_dim, n_dim], F32, kind="ExternalInput")
    b_in = nc.dram_tensor("b", [n_dim], F32, kind="ExternalInput")
    if timing:
        out = nc.dram_tensor("out_scratch", [m_sh, n_dim], F32)
        sink = nc.dram_tensor("out", [P, 512], F32, kind="ExternalOutput")
    else:
        out = nc.dram_tensor("out", [m_sh, n_dim], F32, kind="ExternalOutput")

    NT = 512  # psum free dim (one bank of fp32)
    KT = P  # contraction tile
    m_tiles = m_sh // P
    k_tiles = k_dim // KT
    n_slices = n_dim // NT
    PREP_C = min(1024, k_dim)  # prep chunk of the k axis
    kpc = PREP_C // KT  # k-tiles per prep chunk
    prep_chunks = k_dim // PREP_C

    import contextlib

    with tile.TileContext(nc) as tc:
        with (
            tc.For_i(0, reps, 1) if reps > 1 else contextlib.nullcontext(),
            tc.tile_pool(name="xt", bufs=1) as xt_pool,
            tc.tile_pool(name="const", bufs=1) as const_pool,
        ):
            # SBUF-resident transposed activations: column block kt holds
            # [K=128, M=m_sh] for contraction tile kt.
            xt = xt_pool.tile([P, k_tiles * m_sh], BF16)

            b_bc = const_pool.tile([P, n_dim], F32)
            nc.sync.dma_start(
                b_bc[:], b_in.ap().rearrange("(a n) -> a n", a=1).broadcast_to([P, n_dim])
            )

            # ---- Stage 1: cast-load x, blocked-transpose into xt ----
            # do_prep modes: 1 = production (SWDGE cast-load + transposes into
            # xt); 0 = no prep (memset xt; timing diagnostic); 2 = decoupled
            # prep (full prep into xt, but matmuls read a memset xt2; timing
            # diagnostic); 3 = HWDGE f32 load + DVE cast + transposes into xt.
            xt_mm = xt
            if do_prep == 2:
                xt_mm = xt_pool.tile([P, k_tiles * m_sh], BF16, name="xt2")
                nc.vector.memset(xt_mm[:], 1.0)
            with tc.tile_pool(name="prep", bufs=6) as prep:
              if do_prep == 0:
                # timing diagnostic: fill xt with a constant instead of real x
                nc.vector.memset(xt[:], 1.0)
              if do_prep in (1, 2):
                pending = None
                for c in range(prep_chunks):
                    cs = slice(c * PREP_C, (c + 1) * PREP_C)
                    for mt in range(m_tiles):
                        ms = slice(mt * P, (mt + 1) * P)
                        xbf = prep.tile([P, PREP_C], BF16)
                        # SWDGE cast-during-DMA: f32 HBM -> bf16 SBUF.
                        nc.gpsimd.dma_start(xbf[:], x_in[ms, cs])
                        if pending is not None:
                            nc.sync.dma_start_transpose(*pending)
                        # out[p, j, m] = xbf[m, j*128+p]: one xbar transpose
                        # scatters the kpc k-tile stripes of this m-tile.
                        out3 = (
                            xt[:, c * kpc * m_sh : (c + 1) * kpc * m_sh]
                            .rearrange("p (j m) -> p j m", j=kpc)[
                                :, :, mt * P : (mt + 1) * P
                            ]
                        )
                        pending = (out3, xbf[:])
                nc.sync.dma_start_transpose(*pending)
              if do_prep == 3:
                pending = None
                for c in range(prep_chunks):
                    cs = slice(c * PREP_C, (c + 1) * PREP_C)
                    for mt in range(m_tiles):
                        ms = slice(mt * P, (mt + 1) * P)
                        xin = prep.tile([P, PREP_C], F32)
                        nc.sync.dma_start(xin[:], x_in[ms, cs])
                        if pending is not None:
                            nc.sync.dma_start_transpose(*pending)
                        xbf = prep.tile([P, PREP_C], BF16)
                        nc.vector.tensor_copy(xbf[:], xin[:])
                        out3 = (
                            xt[:, c * kpc * m_sh : (c + 1) * kpc * m_sh]
                            .rearrange("p (j m) -> p j m", j=kpc)[
                                :, :, mt * P : (mt + 1) * P
                            ]
                        )
                        pending = (out3, xbf[:])
                nc.sync.dma_start_transpose(*pending)
            if do_prep == 2:
                # dummy reader so the decoupled xt passes release checks;
                # lands in the last n-slice's bias region (timing-only mode).
                nc.vector.tensor_copy(b_bc[:, n_dim - 512 :], xt[:, :512])

            # ---- Stage 2: main matmul loop ----
            with (
                tc.tile_pool(name="wf", bufs=8) as wf_pool,
                tc.tile_pool(name="wq", bufs=8) as wq_pool,
                tc.tile_pool(name="psum", bufs=8, space="PSUM") as psum_pool,
                tc.tile_pool(name="osb", bufs=4) as out_pool,
            ):
                osb = None
                for ns in range(n_slices):
                    nss = slice(ns * NT, (ns + 1) * NT)
                    psums = [
                        psum_pool.tile([P, NT], F32, name=f"ps_{ns}_{mt}", tag="ps")
                        for mt in range(m_tiles)
                    ]
                    for kt in range(k_tiles):
                        wf = wf_pool.tile([P, NT], F32, name=f"wf_{ns}_{kt}", tag="wf")
                        nc.scalar.dma_start(wf[:], w_in[kt * KT : (kt + 1) * KT, nss])
                        wq = wq_pool.tile([P, NT], BF16, name=f"wq_{ns}_{kt}", tag="wq")
                        nc.scalar.sign(wq[:], wf[:])
                        for mt in range(m_tiles):
                            xo = kt * m_sh + mt * P
                            nc.tensor.matmul(
                                psums[mt][:],
                                xt_mm[:, xo : xo + P],
                                wq[:],
                                start=(kt == 0),
                                stop=(kt == k_tiles - 1),
                            )
                    for mt in range(m_tiles):
                        osb = out_pool.tile([P, NT], F32, name=f"osb_{ns}_{mt}", tag="osb")
                        nc.vector.tensor_add(osb[:], psums[mt][:], b_bc[:, nss])
                        nc.gpsimd.dma_start(out[mt * P : (mt + 1) * P, nss], osb[:])
                if timing:
                    nc.sync.dma_start(sink[:], osb[:])

    nc.compile()
    return nc


_NC_CACHE = {}


def _get_module(m_sh=M_SH, k_dim=N_IN, n_dim=N_UNITS):
    key = (m_sh, k_dim, n_dim)
    if key not in _NC_CACHE:
        _NC_CACHE[key] = build_module(m_sh, k_dim, n_dim)
    return _NC_CACHE[key]


def kernel(x: np.ndarray, W: np.ndarray, b: np.ndarray) -> np.ndarray:
    x = np.ascontiguousarray(np.asarray(x, dtype=np.float32))
    W = np.ascontiguousarray(np.asarray(W, dtype=np.float32))
    b = np.ascontiguousarray(np.asarray(b, dtype=np.float32))
    assert x.shape == (B, N_IN) and W.shape == (N_IN, N_UNITS) and b.shape == (N_UNITS,)

    nc = _get_module()
    in_maps = [
        {"x": x[i * M_SH : (i + 1) * M_SH], "W": W, "b": b} for i in range(N_CORES)
    ]
    res = run_bass_kernel_spmd(nc, in_maps, core_ids=list(range(N_CORES)))
    return np.concatenate(
        [res.results[i]["out"] for i in range(N_CORES)], axis=0
    ).astype(np.float32)


# revision 5
# speedup vs baseline: 1.2096x; 1.2096x over previous
"""Binary-weight dense layer on 8 TRN2 NeuronCores — single-pass bf16, v3.

Computes out = x @ sign(W) + b for x:[8192,4096] f32, W:[4096,4096] f32,
b:[4096] f32, sharded row-wise over x (each core computes a [1024, 4096]
slice of the output; no collectives).

Per-core kernel strategy:
  - single bf16 matmul pass (sign(W) exact in bf16; x rounding ~2^-9 →
    ~1.7e-3 output rel err vs the 2e-2 gate).
  - prep: x tiles are cast f32->bf16 *during* the SWDGE load DMA, then one
    3D-out xbar transpose per [128 x 1024] tile scatters its eight k-tile
    stripes straight into the SBUF-resident lhsT layout (SBUF->SBUF, no
    DRAM staging roundtrip - prep HBM traffic is just the 16 MB x read).
  - engine split so nothing queues behind paced prep triggers: SP ring
    carries b + transposes, ACT ring carries W loads + sign quantization,
    SWDGE carries cast-loads + out writes, DVE does bias-add evictions.
  - main loop: for each n-slice (512 cols) keep 8 PSUM banks (one per
    m-tile) accumulating over all 32 k-tiles.
"""

import sys

if "/opt/trn_rl_repo" not in sys.path:
    sys.path.insert(0, "/opt/trn_rl_repo")

import numpy as np

import concourse.bass as bass
import concourse.mybir as mybir
import concourse.tile as tile
from concourse import bacc
from concourse.bass_utils import run_bass_kernel_spmd

N_CORES = 8
P = 128

B, N_IN, N_UNITS = 8192, 4096, 4096
M_SH = B // N_CORES  # 1024 rows of x per core

F32 = mybir.dt.float32
BF16 = mybir.dt.bfloat16


def build_module(m_sh=M_SH, k_dim=N_IN, n_dim=N_UNITS, reps=1, timing=False, do_prep=1, prep_c=0):
    nc = bacc.Bacc("TRN2", target_bir_lowering=False, debug=False)

    x_in = nc.dram_tensor("x", [m_sh, k_dim], F32, kind="ExternalInput")
    w_in = nc.dram_tensor("W", [k_dim, n_dim], F32, kind="ExternalInput")
    b_in = nc.dram_tensor("b", [n_dim], F32, kind="ExternalInput")
    if timing:
        out = nc.dram_tensor("out_scratch", [m_sh, n_dim], F32)
        sink = nc.dram_tensor("out", [P, 512], F32, kind="ExternalOutput")
    else:
        out = nc.dram_tensor("out", [m_sh, n_dim], F32, kind="ExternalOutput")

    NT = 512  # psum free dim (one bank of fp32)
    KT = P  # contraction tile
    m_tiles = m_sh // P
    k_tiles = k_dim // KT
    n_slices = n_dim // NT
    PREP_C = min(prep_c or 1024, k_dim)  # prep chunk of the k axis
    kpc = PREP_C // KT  # k-tiles per prep chunk
    prep_chunks = k_dim // PREP_C

    import contextlib

    with tile.TileContext(nc) as tc:
        with (
            tc.For_i(0, reps, 1) if reps > 1 else contextlib.nullcontext(),
            tc.tile_pool(name="xt", bufs=1) as xt_pool,
            tc.tile_pool(name="const", bufs=1) as const_pool,
        ):
            # SBUF-resident transposed activations: column block kt holds
            # [K=128, M=m_sh] for contraction tile kt.
            xt = xt_pool.tile([P, k_tiles * m_sh], BF16)

            b_bc = const_pool.tile([P, n_dim], F32)
            nc.sync.dma_start(
                b_bc[:], b_in.ap().rearrange("(a n) -> a n", a=1).broadcast_to([P, n_dim])
            )

            # ---- Stage 1: cast-load x, blocked-transpose into xt ----
            # do_prep modes: 1 = production (SWDGE cast-load + transposes into
            # xt); 0 = no prep (memset xt; timing diagnostic); 2 = decoupled
            # prep (full prep into xt, but matmuls read a memset xt2; timing
            # diagnostic); 3 = HWDGE f32 load + DVE cast + transposes into xt.
            xt_mm = xt
            if do_prep == 2:
                xt_mm = xt_pool.tile([P, k_tiles * m_sh], BF16, name="xt2")
                nc.vector.memset(xt_mm[:], 1.0)
            with tc.tile_pool(name="prep", bufs=3) as prep:
              if do_prep == 0:
                # timing diagnostic: fill xt with a constant instead of real x
                nc.vector.memset(xt[:], 1.0)
              if do_prep in (1, 2):
                pending = None
                for c in range(prep_chunks):
                    cs = slice(c * PREP_C, (c + 1) * PREP_C)
                    for mt in range(m_tiles):
                        ms = slice(mt * P, (mt + 1) * P)
                        xbf = prep.tile([P, PREP_C], BF16)
                        # SWDGE cast-during-DMA: f32 HBM -> bf16 SBUF.
                        nc.gpsimd.dma_start(xbf[:], x_in[ms, cs])
                        if pending is not None:
                            nc.sync.dma_start_transpose(*pending)
                        # out[p, j, m] = xbf[m, j*128+p]: one xbar transpose
                        # scatters the kpc k-tile stripes of this m-tile.
                        out3 = (
                            xt[:, c * kpc * m_sh : (c + 1) * kpc * m_sh]
                            .rearrange("p (j m) -> p j m", j=kpc)[
                                :, :, mt * P : (mt + 1) * P
                            ]
                        )
                        pending = (out3, xbf[:])
                nc.sync.dma_start_transpose(*pending)
              if do_prep == 3:
                pending = None
                for c in range(prep_chunks):
                    cs = slice(c * PREP_C, (c + 1) * PREP_C)
                    for mt in range(m_tiles):
                        ms = slice(mt * P, (mt + 1) * P)
                        xin = prep.tile([P, PREP_C], F32)
                        nc.sync.dma_start(xin[:], x_in[ms, cs])
                        if pending is not None:
                            nc.sync.dma_start_transpose(*pending)
                        xbf = prep.tile([P, PREP_C], BF16)
                        nc.vector.tensor_copy(xbf[:], xin[:])
                        out3 = (
                            xt[:, c * kpc * m_sh : (c + 1) * kpc * m_sh]
                            .rearrange("p (j m) -> p j m", j=kpc)[
                                :, :, mt * P : (mt + 1) * P
                            ]
                        )
                        pending = (out3, xbf[:])
                nc.sync.dma_start_transpose(*pending)
            if do_prep == 2:
                # dummy reader so the decoupled xt passes release checks;
                # lands in the last n-slice's bias region (timing-only mode).
                nc.vector.tensor_copy(b_bc[:, n_dim - 512 :], xt[:, :512])

            # ---- Stage 2: main matmul loop ----
            with (
                tc.tile_pool(name="wf", bufs=4) as wf_pool,
                tc.tile_pool(name="wq", bufs=4) as wq_pool,
                tc.tile_pool(name="psum", bufs=8, space="PSUM") as psum_pool,
                tc.tile_pool(name="osb", bufs=4) as out_pool,
            ):
                osb = None
                for ns in range(n_slices):
                    nss = slice(ns * NT, (ns + 1) * NT)
                    psums = [
                        psum_pool.tile([P, NT], F32, name=f"ps_{ns}_{mt}", tag="ps")
                        for mt in range(m_tiles)
                    ]
                    for kt in range(k_tiles):
                        wf = wf_pool.tile([P, NT], F32, name=f"wf_{ns}_{kt}", tag="wf")
                        nc.scalar.dma_start(wf[:], w_in[kt * KT : (kt + 1) * KT, nss])
                        wq = wq_pool.tile([P, NT], BF16, name=f"wq_{ns}_{kt}", tag="wq")
                        nc.scalar.sign(wq[:], wf[:])
                        for mt in range(m_tiles):
                            xo = kt * m_sh + mt * P
                            nc.tensor.matmul(
                                psums[mt][:],
                                xt_mm[:, xo : xo + P],
                                wq[:],
                                start=(kt == 0),
                                stop=(kt == k_tiles - 1),
                            )
                    for mt in range(m_tiles):
                        osb = out_pool.tile([P, NT], F32, name=f"osb_{ns}_{mt}", tag="osb")
                        nc.vector.tensor_add(osb[:], psums[mt][:], b_bc[:, nss])
                        nc.gpsimd.dma_start(out[mt * P : (mt + 1) * P, nss], osb[:])
                if timing:
                    nc.sync.dma_start(sink[:], osb[:])

    nc.compile()
    return nc


_NC_CACHE = {}


def _get_module(m_sh=M_SH, k_dim=N_IN, n_dim=N_UNITS):
    key = (m_sh, k_dim, n_dim)
    if key not in _NC_CACHE:
        _NC_CACHE[key] = build_module(m_sh, k_dim, n_dim)
    return _NC_CACHE[key]


def kernel(x: np.ndarray, W: np.ndarray, b: np.ndarray) -> np.ndarray:
    x = np.ascontiguousarray(np.asarray(x, dtype=np.float32))
    W = np.ascontiguousarray(np.asarray(W, dtype=np.float32))
    b = np.ascontiguousarray(np.asarray(b, dtype=np.float32))
    assert x.shape == (B, N_IN) and W.shape == (N_IN, N_UNITS) and b.shape == (N_UNITS,)

    nc = _get_module()
    in_maps = [
        {"x": x[i * M_SH : (i + 1) * M_SH], "W": W, "b": b} for i in range(N_CORES)
    ]
    res = run_bass_kernel_spmd(nc, in_maps, core_ids=list(range(N_CORES)))
    return np.concatenate(
        [res.results[i]["out"] for i in range(N_CORES)], axis=0
    ).astype(np.float32)
